# revision 14
# baseline (speedup 1.0000x reference)
"""Trainium2 Bass kernel for nn_DiscoveryNet (pairwise-distance MLP forces).

Math (per batch of N=64 atoms):
  sq[i,j]  = |p_i|^2 + |p_j|^2 - 2 p_i.p_j         (one K=5 matmul per batch)
  r        = rsqrt(max(sq, eps))                    (Quake seed + 2 Newton, DVE)
  dist     = sq * r;  inv_r = min(r, 2) = 1/max(dist,.5)
  invd     = min(r, 100) * offdiag_mask = mask/max(dist,.01)
  feats    = [dist, inv_r, inv_r^6, ^12, ^7, ^13]   (DVE, matrix layout)
  mag      = W3' tanh(W2' tanh(W1' f + b1) + b2)    (flat layout, f32r matmuls)
  w        = mag * invd        (b3 handled via a separate invd-weighted matmul)
  force_i  = p_i * sum_j w_tot[i,j] - sum_j w_tot[i,j] p_j
The heavy MLP runs on flat pair chunks [6or128 x 512]; mag/dist/w are symmetric
in (i,j), and matrix<->flat reshapes go through a DRAM bounce / SBUF DMAs whose
access patterns keep >=256B contiguous runs.

Data parallel over batch: 8 NeuronCores x 64 batches.
"""

import sys

for p in ("/opt/trn_rl_repo",):
    if p not in sys.path:
        sys.path.append(p)

import numpy as np

import concourse.bass as bass
import concourse.tile as tile
import concourse.mybir as mybir
from concourse import bacc
from concourse.bass_utils import run_bass_kernel_spmd

f32 = mybir.dt.float32
f32r = mybir.dt.float32r
bf16 = mybir.dt.bfloat16
i32 = mybir.dt.int32
OP = mybir.AluOpType
AF = mybir.ActivationFunctionType

B, N, D, H = 512, 64, 3, 128
NCORES = 8
BC = B // NCORES        # 64 batches per core
GB = 8                  # batches per group
NG = BC // GB           # 8 groups
CH = 512                # MLP chunk (pairs)
NCHB = (N * N) // CH    # 8 chunks per batch
NF = 6                  # MLP input features


def _build_nc():
    nc = bacc.Bacc(None, target_bir_lowering=False)

    pos = nc.declare_dram_parameter("pos", [BC, N, D], f32, isOutput=False)
    w1 = nc.declare_dram_parameter("w1", [6, H], f32, isOutput=False)
    w2 = nc.declare_dram_parameter("w2", [H, H], f32, isOutput=False)
    w3 = nc.declare_dram_parameter("w3", [H, 32], f32, isOutput=False)
    b1 = nc.declare_dram_parameter("b1", [H, 1], f32, isOutput=False)
    b2 = nc.declare_dram_parameter("b2", [H, 1], f32, isOutput=False)
    b3 = nc.declare_dram_parameter("b3", [N, 1], f32, isOutput=False)
    msk = nc.declare_dram_parameter("msk", [N, N], f32, isOutput=False)
    idn = nc.declare_dram_parameter("idn", [N, N], f32, isOutput=False)
    out = nc.declare_dram_parameter("out", [BC, N, D], f32, isOutput=True)

    with tile.TileContext(nc) as tc:
        with (
            tc.tile_pool(name="const", bufs=1) as cp,
            tc.tile_pool(name="grp", bufs=2) as gp,
            tc.tile_pool(name="chk", bufs=3) as kp,
            tc.tile_pool(name="ps", bufs=1, space=bass.MemorySpace.PSUM) as pp,
            tc.tile_pool(name="psh", bufs=2, space=bass.MemorySpace.PSUM) as pph,
            tc.tile_pool(name="dram", bufs=2, space="DRAM") as dp,
        ):
            # ---- one-time constants ----
            w1s = cp.tile([6, H], f32)
            nc.sync.dma_start(w1s[:], w1[:])
            w2s = cp.tile([H, H], f32)
            nc.sync.dma_start(w2s[:], w2[:])
            w3s = cp.tile([H, 32], f32)
            nc.sync.dma_start(w3s[:], w3[:])
            b1s = cp.tile([H, 1], f32)
            nc.sync.dma_start(b1s[:], b1[:])
            b2s = cp.tile([H, 1], f32)
            nc.sync.dma_start(b2s[:], b2[:])
            b3s = cp.tile([N, 1], f32)
            nc.sync.dma_start(b3s[:], b3[:])
            msks = cp.tile([N, N], f32)
            nc.sync.dma_start(msks[:], msk[:])
            idns = cp.tile([N, N], f32)
            nc.sync.dma_start(idns[:], idn[:])
            ones = cp.tile([1, N * GB], f32)
            nc.vector.memset(ones[:], 1.0)
            ones = cp.tile([1, N * GB], f32)
            nc.vector.memset(ones[:], 1.0)
            w1r = cp.tile([6, H], f32r)
            nc.vector.tensor_copy(w1r[:], w1s[:])
            w2r = cp.tile([H, H], f32r)
            nc.vector.tensor_copy(w2r[:], w2s[:])
            w3r = cp.tile([H, 32], bf16)
            nc.vector.tensor_copy(w3r[:], w3s[:])

            for g in range(NG):
                b0 = g * GB
                # ---- stage A: pos prep ----
                # L1 cols (b,[x,y,z,r2]), L2 cols (b,[x,y,z,1])
                l1 = gp.tile([N, 4 * GB], f32, tag="l1")
                l1v = l1[:].rearrange("p (b c) -> p b c", c=4)
                src_pos = pos[b0 : b0 + GB].rearrange("b a d -> a b d")
                nc.scalar.dma_start(l1v[:, :, 0:3], src_pos)
                l2 = gp.tile([N, 4 * GB], f32, tag="l2")
                l2v = l2[:].rearrange("p (b c) -> p b c", c=4)
                nc.scalar.dma_start(l2v[:, :, 0:3], src_pos)
                nc.vector.memset(l2v[:, :, 3:4], 1.0)

                sq3 = gp.tile([N, 3 * GB], f32, tag="sq3")
                sq3v = sq3[:].rearrange("p (b c) -> p b c", c=3)
                nc.vector.scalar_tensor_tensor(
                    sq3v, l1v[:, :, 0:3], 0.0, l1v[:, :, 0:3], OP.add, OP.mult
                )
                nc.vector.tensor_reduce(
                    l1v[:, :, 3:4], sq3v, mybir.AxisListType.X, OP.add
                )

                # f32r copies of L2 for the force matmuls
                l2r = gp.tile([N, 4 * GB], f32r, tag="l2r")
                nc.vector.tensor_copy(l2r[:], l2[:])
                l2b3 = gp.tile([N, 4 * GB], f32r, tag="l2b3")
                nc.vector.tensor_scalar(l2b3[:], l2[:], b3s[:, 0:1], None, OP.mult)
                l2rv = l2r[:].rearrange("p (b c) -> p b c", c=4)
                l2b3v = l2b3[:].rearrange("p (b c) -> p b c", c=4)

                # L5 rows [x,y,z,r2,1]; R5 rows [-2x,-2y,-2z,1,r2]
                # (engines cannot address partition offsets 1..31, so rows that
                #  land at offsets 3/4 are filled by DMA)
                l5 = gp.tile([5, N * GB], f32, tag="l5")
                r5 = gp.tile([5, N * GB], f32, tag="r5")
                nc.scalar.dma_start(l5[4:5, :], ones[:])
                nc.scalar.dma_start(r5[3:4, :], ones[:])
                for b in range(GB):
                    s = slice(N * b, N * (b + 1))
                    t4b = pp.tile([4, N], f32, tag="t4")
                    nc.tensor.transpose(t4b[:], l1[:, 4 * b : 4 * b + 4], idns[:])
                    nc.vector.tensor_copy(l5[0:4, s], t4b[:])
                    nc.vector.tensor_scalar(
                        r5[0:3, s], t4b[0:3, :], -2.0, None, OP.mult
                    )
                    s4 = gp.tile([4, N], f32, tag="s4")
                    nc.vector.tensor_copy(s4[:], t4b[:])
                    nc.scalar.dma_start(r5[4:5, s], s4[3:4, :])

                # sq matrix per batch (fp32 matmul, K=5)
                sq = pp.tile([N, N * GB], f32, tag="sq")
                for b in range(GB):
                    s = slice(N * b, N * (b + 1))
                    nc.tensor.matmul(sq[:, s], l5[:, s], r5[:, s], start=True, stop=True)

                # ---- features on DVE (matrix layout [64, 512]) ----
                f7 = gp.tile([N, GB * NF * N], f32r, tag="f7")
                f7v = f7[:].rearrange("p (b f j) -> p b f j", f=NF, j=N)
                invd = gp.tile([N, GB * N], f32r, tag="invd")
                invdv = invd[:].rearrange("p (b j) -> p b j", j=N)

                def fsl(fi):
                    return f7v[:, :, fi, :]

                mc = gp.tile([N, N * GB], f32, tag="mc")
                rs = gp.tile([N, N * GB], f32, tag="rs")
                t1 = gp.tile([N, N * GB], f32, tag="t1")
                t2 = gp.tile([N, N * GB], f32, tag="t2")
                mcv = mc[:].rearrange("p (b j) -> p b j", j=N)
                rsv = rs[:].rearrange("p (b j) -> p b j", j=N)
                t1v = t1[:].rearrange("p (b j) -> p b j", j=N)
                t2v = t2[:].rearrange("p (b j) -> p b j", j=N)

                nc.vector.tensor_scalar(mc[:], sq[:], 1e-12, None, OP.max)
                mci = mc[:].bitcast(i32)
                rsi = rs[:].bitcast(i32)
                nc.vector.tensor_scalar(rsi, mci, 1, None, OP.logical_shift_right)
                nc.vector.tensor_scalar(rsi, rsi, -1, 0x5F3759DF, OP.mult, OP.add)
                for _ in range(2):
                    nc.vector.tensor_tensor(t1[:], rs[:], rs[:], OP.mult)
                    nc.vector.scalar_tensor_tensor(
                        t1[:], t1[:], -0.5, mc[:], OP.mult, OP.mult
                    )
                    nc.vector.scalar_tensor_tensor(
                        rs[:], t1[:], 1.5, rs[:], OP.add, OP.mult
                    )
                # dist
                nc.vector.tensor_tensor(fsl(0), mcv, rsv, OP.mult)
                # inv_r
                nc.vector.tensor_scalar(fsl(1), rsv, 2.0, None, OP.min)
                # powers: t1=i2, t2=i4, f2=i6, f3=i12, f4=i7, f5=i13
                ivr = fsl(1).bitcast(f32)
                nc.vector.tensor_tensor(t1v, ivr, ivr, OP.mult)
                nc.vector.tensor_tensor(t2v, t1v, t1v, OP.mult)
                nc.vector.tensor_tensor(fsl(2), t2v, t1v, OP.mult)
                i6 = fsl(2).bitcast(f32)
                nc.vector.tensor_tensor(fsl(3), i6, i6, OP.mult)
                nc.vector.tensor_tensor(fsl(4), i6, ivr, OP.mult)
                i12 = fsl(3).bitcast(f32)
                nc.vector.tensor_tensor(fsl(5), i12, ivr, OP.mult)
                # invd = min(rs, 100) * mask
                mrep = msks[:].rearrange("p (one j) -> p one j", one=1).broadcast_to(
                    (N, GB, N)
                )
                nc.vector.scalar_tensor_tensor(
                    invdv, rsv, 100.0, mrep, OP.min, OP.mult
                )

                # ---- flatten via DRAM bounce ----
                fd = dp.tile([GB, NF, N, N], f32r, tag="fd")
                nc.sync.dma_start(
                    fd[:].rearrange("b f i j -> i (b f) j"),
                    f7[:].rearrange("p (b f j) -> p (b f) j", f=NF, j=N),
                )

                # ---- stage B: MLP over 512-pair chunks ----
                outg = gp.tile([N, 3 * GB], f32, tag="outg")
                outgv = outg[:].rearrange("p (b c) -> p b c", c=3)
                for b in range(GB):
                    ftb = kp.tile([6, N * N], f32r, tag="ftb")
                    nc.sync.dma_start(
                        ftb[:], fd[b].rearrange("f i j -> f (i j)")
                    )
                    m64 = kp.tile([N, N], f32, tag="m64")
                    for r in range(NCHB):
                        if r % 3 == 0:
                            mag = pp.tile([96, CH], f32, tag="mag")
                        h1 = pph.tile([H, CH], f32, tag="h1")
                        nc.tensor.matmul(
                            h1[:], w1r[:], ftb[:, CH * r : CH * (r + 1)],
                            start=True, stop=True,
                        )
                        h1s = kp.tile([H, CH], f32r, tag="h1s")
                        nc.scalar.activation(
                            h1s[:], h1[:], AF.Tanh, bias=b1s[:, 0:1], scale=1.0
                        )
                        h2 = pph.tile([H, CH], f32, tag="h2")
                        nc.tensor.matmul(h2[:], w2r[:], h1s[:], start=True, stop=True)
                        h2s = kp.tile([H, CH], bf16, tag="h2s")
                        nc.scalar.activation(
                            h2s[:], h2[:], AF.Tanh, bias=b2s[:, 0:1], scale=1.0
                        )
                        c = r % 3
                        nc.tensor.matmul(
                            mag[32 * c : 32 * (c + 1), :], w3r[:], h2s[:],
                            start=True, stop=True,
                        )
                        if c == 2 or r == NCHB - 1:
                            nrow = c + 1
                            stg3 = kp.tile([96, CH], f32, tag="stg3")
                            nc.vector.tensor_copy(
                                stg3[0 : 32 * nrow, :], mag[0 : 32 * nrow, :]
                            )
                            k3 = r // 3
                            nc.sync.dma_start(
                                m64[8 * 3 * k3 : 8 * 3 * k3 + 8 * nrow, :],
                                stg3[:].rearrange("(a e) (il j) -> a e il j", e=32, j=N)[
                                    0:nrow, 0, :, :
                                ],
                            )
                    # w = mag*invd (matrix layout), then force
                    w64 = kp.tile([N, N], f32r, tag="w64")
                    nc.vector.tensor_tensor(
                        w64[:], m64[:], invdv[:, b, :].bitcast(f32), OP.mult
                    )
                    fp = pp.tile([4, N], f32, tag="fp")
                    nc.tensor.matmul(
                        fp[:], l2rv[:, b, :], w64[:], start=True, stop=False
                    )
                    nc.tensor.matmul(
                        fp[:],
                        l2b3v[:, b, :],
                        invdv[:, b, :],
                        start=False,
                        stop=True,
                    )
                    fps = kp.tile([4, N], f32, tag="fps")
                    nc.vector.tensor_copy(fps[:], fp[:])
                    ft4 = pp.tile([N, 4], f32, tag="t4")
                    nc.tensor.transpose(ft4[:], fps[:], idns[0:4, 0:4])
                    nc.vector.scalar_tensor_tensor(
                        outgv[:, b, :],
                        l2v[:, b, 0:3],
                        ft4[:, 3:4],
                        ft4[:, 0:3],
                        OP.mult,
                        OP.subtract,
                    )
                nc.scalar.dma_start(
                    out[b0 : b0 + GB].rearrange("b a d -> a b d"), outg[:].rearrange(
                        "p (b c) -> p b c", c=3
                    )
                )

    nc.compile()
    return nc


_NC_CACHE = {}


def _get_nc():
    if "nc" not in _NC_CACHE:
        _NC_CACHE["nc"] = _build_nc()
    return _NC_CACHE["nc"]


def kernel(pos_scaled, W1, b1, W2, b2, W3, b3):
    nc = _get_nc()
    pos_scaled = np.ascontiguousarray(np.asarray(pos_scaled, dtype=np.float32))
    w1 = np.ascontiguousarray(np.asarray(W1, dtype=np.float32))
    w2 = np.ascontiguousarray(np.asarray(W2, dtype=np.float32))
    w3 = np.ascontiguousarray(np.tile(np.asarray(W3, dtype=np.float32).reshape(H, 1), (1, 32)))
    b1c = np.ascontiguousarray(np.asarray(b1, dtype=np.float32).reshape(H, 1))
    b2c = np.ascontiguousarray(np.asarray(b2, dtype=np.float32).reshape(H, 1))
    b3c = np.full((N, 1), float(np.asarray(b3).reshape(-1)[0]), dtype=np.float32)
    mask = (1.0 - np.eye(N, dtype=np.float32)).astype(np.float32)
    ident = np.eye(N, dtype=np.float32)

    in_maps = []
    for c in range(NCORES):
        in_maps.append(
            {
                "pos": pos_scaled[c * BC : (c + 1) * BC],
                "w1": w1,
                "w2": w2,
                "w3": w3,
                "b1": b1c,
                "b2": b2c,
                "b3": b3c,
                "msk": mask,
                "idn": ident,
            }
        )
    res = run_bass_kernel_spmd(nc, in_maps, core_ids=list(range(NCORES)))
    return np.concatenate([res.results[c]["out"] for c in range(NCORES)], axis=0)
